# revision 28
# baseline (speedup 1.0000x reference)
"""Aitchison-Aitken categorical kernel on 8 TRN2 NeuronCores.

Math (reference, NUM_LEVELS=4, n_feat=64):
    w_f     = log(1-h_f) - log(h_f/3)
    base    = sum_f log(h_f/3) - sum_f log(h_f) = -64*log(3)   (data independent)
    match   = sum_f w_f * 1[test_if == train_jf]           ([n_test, n_train])
    ld      = match + base
    out     = rowmax(ld) * exp(ld - rowmax(ld))

Device algorithm (per core, data-parallel over test rows):
  - encode test/train as fp16 level-indicator matrices, K = 4*64 = 256
    (2 K-tiles of 128: [lvl0|lvl1], [lvl2|lvl3], feature-duplicated halves);
    weights w folded into the test-side encoding
  - match tile: 2x fp16 matmul accumulation in PSUM (TensorE)
  - ebuf = exp(match + base) via ScalarE straight from PSUM -> bf16
    (no max subtraction needed: exp(ld) spans ~[e^-150, e^41], f32/bf16-safe)
  - row max of ebuf via a bf16 tensor_tensor max fold ladder (2x DVE mode)
  - c = ln(m)/m  (identity: m = e^maxld  =>  c*e^ld = maxld*e^{ld-maxld})
  - out = ebuf * c  (tensor_scalar per-partition; some tiles on ScalarE)

Sharding: test_Xs rows across 8 cores; bandwidths/train_Xs replicated;
out [1024, 8192] local per core, host-concatenated.
"""
import numpy as np
from contextlib import ExitStack

from concourse import bacc, hw_specs, mybir, masks, tile
from concourse.bass_utils import run_bass_kernel_spmd

f32 = mybir.dt.float32
f16 = mybir.dt.float16
bf16 = mybir.dt.bfloat16
ACTF = mybir.ActivationFunctionType
ALU = mybir.AluOpType

N_CORES = 8
N_TEST, N_TRAIN, N_FEAT = 8192, 8192, 64
M_LOC = N_TEST // N_CORES          # 1024 test rows per core
P = 128                            # partitions
M_TILES = M_LOC // P               # 8
NT = 512                           # train cols per matmul (one PSUM bank)
QCOL = 2048                       # psum tile / epilogue chunk (4 banks)
NQ = N_TRAIN // QCOL               # 4
BASE = float(-N_FEAT * np.log(3.0))

# which of the 8 M-tiles get their final multiply on ScalarE
MULT_ON_ACT = set()
GP_FOLD = False

# ---------------------------------------------------------------------------
# Activation-table patch: both Exp and Ln are needed; the stock fixpoint pass
# resolves each to the first table set containing it (exp_and_others vs
# natural_log), reloading the ACT tables (~1.3us) on every switch. Hide
# Exp/Ln from every other set so both resolve to natural_log_exp_and_others
# at its true act_func_set_id (list order/indices preserved).
_COMBINED = "natural_log_exp_and_others"
_orig_get_tables = hw_specs.get_activation_tables


def _patched_tables(module_arch):
    d = _orig_get_tables(module_arch)
    if _COMBINED not in d:
        return d
    hide = {ACTF.Exp, ACTF.Ln}
    return {k: (set(v) if k == _COMBINED else set(v) - hide)
            for k, v in d.items()}


hw_specs.get_activation_tables = _patched_tables
bacc.get_activation_tables = _patched_tables   # bacc imported its own ref
# ---------------------------------------------------------------------------


def _build():
    nc = bacc.Bacc(None, target_bir_lowering=False)
    bw_ext = nc.declare_dram_parameter("bandwidths", [N_FEAT], f32, isOutput=False)
    test_ext = nc.declare_dram_parameter("test_Xs", [M_LOC, N_FEAT], f32, isOutput=False)
    train_ext = nc.declare_dram_parameter("train_Xs", [N_TRAIN, N_FEAT], f32, isOutput=False)
    out_ext = nc.declare_dram_parameter("out", [M_LOC, N_TRAIN], f32, isOutput=True)

    with tile.TileContext(nc) as tc, ExitStack() as ctx:
        const = ctx.enter_context(tc.tile_pool(name="const", bufs=1))
        enc = ctx.enter_context(tc.tile_pool(name="enc", bufs=1))
        stats = ctx.enter_context(tc.tile_pool(name="stats", bufs=4))
        fold_p = ctx.enter_context(tc.tile_pool(name="fold", bufs=1))
        ebuf_pool = ctx.enter_context(tc.tile_pool(name="ebuf", bufs=3))
        out_pool = ctx.enter_context(tc.tile_pool(name="obuf", bufs=2))

        # ---- constants -------------------------------------------------
        ident16 = const.tile([P, P], f16)
        masks.make_identity(nc, ident16[:])

        one_t = const.tile([N_FEAT, 1], f32)
        nc.vector.memset(one_t[:], 1.0)
        base_t = const.tile([P, 1], f32)
        nc.vector.memset(base_t[:], BASE)
        lvlA = const.tile([P, 1], f32)
        nc.vector.memset(lvlA[0:64, :], 0.0)
        nc.vector.memset(lvlA[64:128, :], 1.0)
        lvlB = const.tile([P, 1], f32)
        nc.vector.memset(lvlB[0:64, :], 2.0)
        nc.vector.memset(lvlB[64:128, :], 3.0)

        # ---- w vector from bandwidths ---------------------------------
        bw = const.tile([N_FEAT, 1], f32)
        nc.sync.dma_start(out=bw[:], in_=bw_ext[:].rearrange("(f o) -> f o", o=1))
        lt = const.tile([N_FEAT, 1], f32)   # log(1 - h)
        nc.scalar.activation(lt[:], bw[:], ACTF.Ln, bias=one_t[:], scale=-1.0)
        w2 = const.tile([P, 1], f32)
        lf = const.tile([N_FEAT, 1], f32)   # log(h/3)
        nc.scalar.activation(lf[:], bw[:], ACTF.Ln, scale=1.0 / 3.0)
        nc.vector.tensor_tensor(w2[0:64, :], lt[:], lf[:], op=ALU.subtract)
        nc.sync.dma_start(out=w2[64:128, :], in_=w2[0:64, :])

        # ---- encode: transpose + level indicators ----------------------
        sencA = enc.tile([P, N_TRAIN], f16)
        sencB = enc.tile([P, N_TRAIN], f16)
        tencA = enc.tile([P, M_LOC], f16)
        tencB = enc.tile([P, M_LOC], f16)

        with tc.tile_pool(name="prep", bufs=3) as prep, \
             tc.tile_pool(name="natp", bufs=1) as natp, \
             tc.tile_pool(name="prep_ps", bufs=2, space="PSUM") as prep_ps:

            def transpose_encode(src_ext, n_rows, dstA, dstB, wmul):
                n_ch = n_rows // P              # 128-row chunks
                # feature-duplicated transposed fp16 values [128, n_rows]
                dstT = natp.tile([P, n_rows], f16, tag=f"dstT_{n_rows}")
                # stage + cast natural-layout fp16 copy, 2048 rows at a time
                nat16 = natp.tile([P, n_ch * N_FEAT], f16, tag=f"nat16_{n_rows}")
                row0 = 0
                groups = [1, 1, 2] + [4] * ((n_ch - 4) // 4) if n_ch > 4 else [n_ch]
                for gch in groups:
                    rows = gch * P
                    cpp = gch * N_FEAT
                    col0 = row0 * N_FEAT // P
                    stage = prep.tile([P, cpp], f32, tag="stage")
                    nc.sync.dma_start(
                        out=stage[:].rearrange("p (c f) -> p c f", f=N_FEAT),
                        in_=src_ext[row0:row0 + rows, :].rearrange(
                            "(c p) f -> p c f", p=P))
                    nc.vector.tensor_copy(nat16[:, col0:col0 + cpp], stage[:])
                    row0 += rows
                n_grp = n_ch // 4               # 4 chunks -> one [128, 512] psum
                for g in range(n_grp):
                    pt = prep_ps.tile([P, NT], f16)
                    for j in range(4):
                        c = g * 4 + j
                        cs = slice(c * N_FEAT, (c + 1) * N_FEAT)
                        nc.tensor.transpose(pt[0:64, j * P:(j + 1) * P],
                                            nat16[:, cs], ident16[:])
                        nc.tensor.transpose(pt[64:128, j * P:(j + 1) * P],
                                            nat16[:, cs], ident16[:],
                                            tile_position=(0, 64))
                    # evacuate psum (both duplicated halves at once)
                    nc.scalar.activation(dstT[:, g * NT:(g + 1) * NT], pt[:],
                                         ACTF.Copy, bias=0.0, scale=1.0)
                # encode in 512-col chunks
                n_q = (n_rows + NT - 1) // NT
                for q in range(n_q):
                    s = slice(q * NT, min((q + 1) * NT, n_rows))
                    if wmul is None:
                        nc.vector.tensor_scalar(dstA[:, s], dstT[:, s], lvlA[:], None,
                                                op0=ALU.is_equal)
                        nc.vector.tensor_scalar(dstB[:, s], dstT[:, s], lvlB[:], None,
                                                op0=ALU.is_equal)
                    else:
                        nc.vector.tensor_scalar(dstA[:, s], dstT[:, s], lvlA[:], wmul[:],
                                                op0=ALU.is_equal, op1=ALU.mult)
                        nc.vector.tensor_scalar(dstB[:, s], dstT[:, s], lvlB[:], wmul[:],
                                                op0=ALU.is_equal, op1=ALU.mult)

            transpose_encode(test_ext, M_LOC, tencA, tencB, w2)
            transpose_encode(train_ext, N_TRAIN, sencA, sencB, None)

        # ---- main loop --------------------------------------------------
        with tc.tile_pool(name="mm_ps", bufs=3, space="PSUM") as mm_ps:
            for m in range(M_TILES):
                ms = slice(m * P, (m + 1) * P)
                ebuf = ebuf_pool.tile([P, N_TRAIN], bf16)
                fa = fold_p.tile([P, N_TRAIN // 4], bf16, tag="fa")
                fb = fold_p.tile([P, N_TRAIN // 4], bf16, tag="fb")
                PS_COL = 1024
                for q in range(N_TRAIN // PS_COL):
                    ps = mm_ps.tile([P, PS_COL], f32)
                    # stationary-reuse order: A,A then B,B
                    for j in range(2):
                        n = q * 2 + j
                        nc.tensor.matmul(ps[:, j * NT:(j + 1) * NT], tencA[:, ms],
                                         sencA[:, n * NT:(n + 1) * NT],
                                         start=True, stop=False)
                    for j in range(2):
                        n = q * 2 + j
                        nc.tensor.matmul(ps[:, j * NT:(j + 1) * NT], tencB[:, ms],
                                         sencB[:, n * NT:(n + 1) * NT],
                                         start=False, stop=True)
                    qs = slice(q * PS_COL, (q + 1) * PS_COL)
                    nc.scalar.activation(ebuf[:, qs], ps[:], ACTF.Exp,
                                         bias=base_t[:], scale=1.0)
                    # first-level fold as soon as half the chunks are ready
                    fold_eng = nc.gpsimd if GP_FOLD else nc.vector
                    if q == 3:
                        fold_eng.tensor_tensor(fa[:], ebuf[:, 0:QCOL],
                                               ebuf[:, QCOL:2 * QCOL], op=ALU.max)
                    elif q == 7:
                        fold_eng.tensor_tensor(fb[:], ebuf[:, 2 * QCOL:3 * QCOL],
                                               ebuf[:, 3 * QCOL:], op=ALU.max)
                # fold ladder tail (2x bf16 TT mode, ping-pong, no aliasing)
                fc = fold_p.tile([P, N_TRAIN // 4], bf16, tag="fc")
                nc.vector.tensor_tensor(fc[:], fa[:], fb[:], op=ALU.max)
                nc.vector.tensor_tensor(fa[:, 0:1024], fc[:, 0:1024], fc[:, 1024:],
                                        op=ALU.max)
                nc.vector.tensor_tensor(fb[:, 0:512], fa[:, 0:512], fa[:, 512:1024],
                                        op=ALU.max)
                nc.vector.tensor_tensor(fc[:, 0:256], fb[:, 0:256], fb[:, 256:512],
                                        op=ALU.max)
                mm_t = stats.tile([P, 1], f32)     # m = max(ebuf) (exact in bf16)
                nc.vector.tensor_reduce(mm_t[:], fc[:, 0:256],
                                        axis=mybir.AxisListType.X, op=ALU.max)
                # c = ln(m)/m
                lnm = stats.tile([P, 1], f32)
                nc.scalar.activation(lnm[:], mm_t[:], ACTF.Ln)
                rec = stats.tile([P, 1], f32)
                nc.vector.reciprocal(rec[:], mm_t[:])
                cvec = stats.tile([P, 1], f32)
                nc.vector.tensor_tensor(cvec[:], lnm[:], rec[:], op=ALU.mult)

                obuf = out_pool.tile([P, N_TRAIN], f32)
                for h in range(2):
                    hs = slice(h * (N_TRAIN // 2), (h + 1) * (N_TRAIN // 2))
                    if m in MULT_ON_ACT:
                        nc.scalar.activation(obuf[:, hs], ebuf[:, hs], ACTF.Copy,
                                             bias=0.0, scale=cvec[:])
                    else:
                        nc.vector.tensor_scalar(obuf[:, hs], ebuf[:, hs], cvec[:],
                                                None, op0=ALU.mult)
                    nc.sync.dma_start(out=out_ext[ms, hs], in_=obuf[:, hs])

    nc.compile()
    return nc


_NC = None


def _get_nc():
    global _NC
    if _NC is None:
        _NC = _build()
    return _NC


def kernel(bandwidths, test_Xs, train_Xs):
    bandwidths = np.ascontiguousarray(bandwidths, dtype=np.float32)
    test_Xs = np.ascontiguousarray(test_Xs, dtype=np.float32)
    train_Xs = np.ascontiguousarray(train_Xs, dtype=np.float32)

    nc = _get_nc()
    in_maps = [
        {
            "bandwidths": bandwidths,
            "test_Xs": np.ascontiguousarray(test_Xs[i * M_LOC:(i + 1) * M_LOC]),
            "train_Xs": train_Xs,
        }
        for i in range(N_CORES)
    ]
    res = run_bass_kernel_spmd(nc, in_maps, core_ids=list(range(N_CORES)))
    return np.concatenate([r["out"] for r in res.results], axis=0)


if __name__ == "__main__":
    rng = np.random.default_rng(0)
    h = rng.uniform(0.05, 0.5, N_FEAT).astype(np.float32)
    t = rng.integers(0, 4, (N_TEST, N_FEAT)).astype(np.float32)
    s = rng.integers(0, 4, (N_TRAIN, N_FEAT)).astype(np.float32)
    out = kernel(bandwidths=h, test_Xs=t, train_Xs=s)
    print(out.shape, out.dtype)


# revision 30
# speedup vs baseline: 1.3453x; 1.3453x over previous
"""Aitchison-Aitken categorical kernel on 8 TRN2 NeuronCores.

Math (reference, NUM_LEVELS=4, n_feat=64):
    w_f     = log(1-h_f) - log(h_f/3)
    base    = sum_f log(h_f/3) - sum_f log(h_f) = -64*log(3)   (data independent)
    match   = sum_f w_f * 1[test_if == train_jf]           ([n_test, n_train])
    ld      = match + base
    out     = rowmax(ld) * exp(ld - rowmax(ld))

Device algorithm (per core, data-parallel over test rows):
  - encode test/train as fp16 level-indicator matrices, K = 4*64 = 256
    (2 K-tiles of 128: [lvl0|lvl1], [lvl2|lvl3], feature-duplicated halves);
    weights w folded into the test-side encoding
  - match tile: 2x fp16 matmul accumulation in PSUM (TensorE)
  - ebuf = exp(match + base) via ScalarE straight from PSUM -> bf16
    (no max subtraction needed: exp(ld) spans ~[e^-150, e^41], f32/bf16-safe)
  - row max of ebuf via a bf16 tensor_tensor max fold ladder (2x DVE mode)
  - c = ln(m)/m  (identity: m = e^maxld  =>  c*e^ld = maxld*e^{ld-maxld})
  - out = ebuf * c  (tensor_scalar per-partition; some tiles on ScalarE)

Sharding: test_Xs rows across 8 cores; bandwidths/train_Xs replicated;
out [1024, 8192] local per core, host-concatenated.
"""
import numpy as np
from contextlib import ExitStack

from concourse import bacc, hw_specs, mybir, masks, tile
from concourse.bass_utils import run_bass_kernel_spmd

f32 = mybir.dt.float32
f16 = mybir.dt.float16
bf16 = mybir.dt.bfloat16
ACTF = mybir.ActivationFunctionType
ALU = mybir.AluOpType

N_CORES = 8
N_TEST, N_TRAIN, N_FEAT = 8192, 8192, 64
M_LOC = N_TEST // N_CORES          # 1024 test rows per core
P = 128                            # partitions
M_TILES = M_LOC // P               # 8
NT = 512                           # train cols per matmul (one PSUM bank)
QCOL = 2048                       # psum tile / epilogue chunk (4 banks)
NQ = N_TRAIN // QCOL               # 4
BASE = float(-N_FEAT * np.log(3.0))

# which of the 8 M-tiles get their final multiply on ScalarE
MULT_ON_ACT = set()
GP_FOLD = False

# ---------------------------------------------------------------------------
# Activation-table patch: both Exp and Ln are needed; the stock fixpoint pass
# resolves each to the first table set containing it (exp_and_others vs
# natural_log), reloading the ACT tables (~1.3us) on every switch. Hide
# Exp/Ln from every other set so both resolve to natural_log_exp_and_others
# at its true act_func_set_id (list order/indices preserved).
_COMBINED = "natural_log_exp_and_others"
_orig_get_tables = hw_specs.get_activation_tables


def _patched_tables(module_arch):
    d = _orig_get_tables(module_arch)
    if _COMBINED not in d:
        return d
    hide = {ACTF.Exp, ACTF.Ln}
    return {k: (set(v) if k == _COMBINED else set(v) - hide)
            for k, v in d.items()}


hw_specs.get_activation_tables = _patched_tables
bacc.get_activation_tables = _patched_tables   # bacc imported its own ref
# ---------------------------------------------------------------------------


def _build():
    nc = bacc.Bacc(None, target_bir_lowering=False)
    bw_ext = nc.declare_dram_parameter("bandwidths", [N_FEAT], f32, isOutput=False)
    test_ext = nc.declare_dram_parameter("test_Xs", [M_LOC, N_FEAT], f32, isOutput=False)
    train_ext = nc.declare_dram_parameter("train_Xs", [N_TRAIN, N_FEAT], f32, isOutput=False)
    out_ext = nc.declare_dram_parameter("out", [M_LOC, N_TRAIN], f16, isOutput=True)

    with tile.TileContext(nc) as tc, ExitStack() as ctx:
        const = ctx.enter_context(tc.tile_pool(name="const", bufs=1))
        mm_ps = ctx.enter_context(tc.tile_pool(name="mm_ps", bufs=3, space="PSUM"))
        enc = ctx.enter_context(tc.tile_pool(name="enc", bufs=1))
        stats = ctx.enter_context(tc.tile_pool(name="stats", bufs=4))
        fold_p = ctx.enter_context(tc.tile_pool(name="fold", bufs=1))
        ebuf_pool = ctx.enter_context(tc.tile_pool(name="ebuf", bufs=3))
        out_pool = ctx.enter_context(tc.tile_pool(name="obuf", bufs=2))

        # ---- constants -------------------------------------------------
        ident16 = const.tile([P, P], f16)
        masks.make_identity(nc, ident16[:])

        one_t = const.tile([N_FEAT, 1], f32)
        nc.vector.memset(one_t[:], 1.0)
        base_t = const.tile([P, 1], f32)
        nc.vector.memset(base_t[:], BASE)
        lvlA = const.tile([P, 1], f32)
        nc.vector.memset(lvlA[0:64, :], 0.0)
        nc.vector.memset(lvlA[64:128, :], 1.0)
        lvlB = const.tile([P, 1], f32)
        nc.vector.memset(lvlB[0:64, :], 2.0)
        nc.vector.memset(lvlB[64:128, :], 3.0)

        # ---- w vector from bandwidths ---------------------------------
        bw = const.tile([N_FEAT, 1], f32)
        nc.sync.dma_start(out=bw[:], in_=bw_ext[:].rearrange("(f o) -> f o", o=1))
        lt = const.tile([N_FEAT, 1], f32)   # log(1 - h)
        nc.scalar.activation(lt[:], bw[:], ACTF.Ln, bias=one_t[:], scale=-1.0)
        w2 = const.tile([P, 1], f32)
        lf = const.tile([N_FEAT, 1], f32)   # log(h/3)
        nc.scalar.activation(lf[:], bw[:], ACTF.Ln, scale=1.0 / 3.0)
        nc.vector.tensor_tensor(w2[0:64, :], lt[:], lf[:], op=ALU.subtract)
        nc.sync.dma_start(out=w2[64:128, :], in_=w2[0:64, :])

        # ---- encode: transpose + level indicators ----------------------
        sencA = enc.tile([P, N_TRAIN], f16)
        sencB = enc.tile([P, N_TRAIN], f16)
        tencA = enc.tile([P, M_LOC], f16)
        tencB = enc.tile([P, M_LOC], f16)

        with tc.tile_pool(name="prep", bufs=3) as prep, \
             tc.tile_pool(name="natp", bufs=1) as natp, \
             tc.tile_pool(name="prep_ps", bufs=2, space="PSUM") as prep_ps:

            def transpose_encode(src_ext, n_rows, dstA, dstB, wmul):
                n_ch = n_rows // P              # 128-row chunks
                # feature-duplicated transposed fp16 values [128, n_rows]
                dstT = natp.tile([P, n_rows], f16, tag=f"dstT_{n_rows}")
                # stage + cast natural-layout fp16 copy, 2048 rows at a time
                nat16 = natp.tile([P, n_ch * N_FEAT], f16, tag=f"nat16_{n_rows}")
                row0 = 0
                groups = [1, 1, 2] + [4] * ((n_ch - 4) // 4) if n_ch > 4 else [n_ch]
                for gch in groups:
                    rows = gch * P
                    cpp = gch * N_FEAT
                    col0 = row0 * N_FEAT // P
                    stage = prep.tile([P, cpp], f32, tag="stage")
                    nc.sync.dma_start(
                        out=stage[:].rearrange("p (c f) -> p c f", f=N_FEAT),
                        in_=src_ext[row0:row0 + rows, :].rearrange(
                            "(c p) f -> p c f", p=P))
                    nc.vector.tensor_copy(nat16[:, col0:col0 + cpp], stage[:])
                    row0 += rows
                n_grp = n_ch // 4               # 4 chunks -> one [64, 512] psum
                for g in range(n_grp):
                    pt = prep_ps.tile([64, NT], f16)
                    for j in range(4):
                        c = g * 4 + j
                        cs = slice(c * N_FEAT, (c + 1) * N_FEAT)
                        nc.tensor.transpose(pt[:, j * P:(j + 1) * P],
                                            nat16[:, cs], ident16[:])
                    # evacuate psum -> fp16 top half
                    nc.scalar.activation(dstT[0:64, g * NT:(g + 1) * NT], pt[:],
                                         ACTF.Copy, bias=0.0, scale=1.0)
                    # duplicate to bottom half (vector-engine DMA queue)
                    gs = slice(g * NT, (g + 1) * NT)
                    nc.gpsimd.dma_start(out=dstT[64:128, gs], in_=dstT[0:64, gs])
                # encode in 512-col chunks
                n_q = (n_rows + NT - 1) // NT
                for q in range(n_q):
                    s = slice(q * NT, min((q + 1) * NT, n_rows))
                    if wmul is None:
                        nc.vector.tensor_scalar(dstA[:, s], dstT[:, s], lvlA[:], None,
                                                op0=ALU.is_equal)
                        nc.vector.tensor_scalar(dstB[:, s], dstT[:, s], lvlB[:], None,
                                                op0=ALU.is_equal)
                    else:
                        nc.vector.tensor_scalar(dstA[:, s], dstT[:, s], lvlA[:], wmul[:],
                                                op0=ALU.is_equal, op1=ALU.mult)
                        nc.vector.tensor_scalar(dstB[:, s], dstT[:, s], lvlB[:], wmul[:],
                                                op0=ALU.is_equal, op1=ALU.mult)

            transpose_encode(test_ext, M_LOC, tencA, tencB, w2)
            transpose_encode(train_ext, N_TRAIN, sencA, sencB, None)

        # ---- main loop --------------------------------------------------
        if True:
            for m in range(M_TILES):
                ms = slice(m * P, (m + 1) * P)
                ebuf = ebuf_pool.tile([P, N_TRAIN], bf16)
                fa = fold_p.tile([P, N_TRAIN // 4], bf16, tag="fa")
                fb = fold_p.tile([P, N_TRAIN // 4], bf16, tag="fb")
                PS_COL = 1024
                for q in range(N_TRAIN // PS_COL):
                    ps = mm_ps.tile([P, PS_COL], f32)
                    # stationary-reuse order: A,A then B,B
                    for j in range(2):
                        n = q * 2 + j
                        nc.tensor.matmul(ps[:, j * NT:(j + 1) * NT], tencA[:, ms],
                                         sencA[:, n * NT:(n + 1) * NT],
                                         start=True, stop=False)
                    for j in range(2):
                        n = q * 2 + j
                        nc.tensor.matmul(ps[:, j * NT:(j + 1) * NT], tencB[:, ms],
                                         sencB[:, n * NT:(n + 1) * NT],
                                         start=False, stop=True)
                    qs = slice(q * PS_COL, (q + 1) * PS_COL)
                    nc.scalar.activation(ebuf[:, qs], ps[:], ACTF.Exp,
                                         bias=base_t[:], scale=1.0)
                    # first-level fold as soon as half the chunks are ready
                    fold_eng = nc.gpsimd if GP_FOLD else nc.vector
                    if q == 3:
                        fold_eng.tensor_tensor(fa[:], ebuf[:, 0:QCOL],
                                               ebuf[:, QCOL:2 * QCOL], op=ALU.max)
                    elif q == 7:
                        fold_eng.tensor_tensor(fb[:], ebuf[:, 2 * QCOL:3 * QCOL],
                                               ebuf[:, 3 * QCOL:], op=ALU.max)
                # fold ladder tail (2x bf16 TT mode, ping-pong, no aliasing)
                fc = fold_p.tile([P, N_TRAIN // 4], bf16, tag="fc")
                nc.vector.tensor_tensor(fc[:], fa[:], fb[:], op=ALU.max)
                nc.vector.tensor_tensor(fa[:, 0:1024], fc[:, 0:1024], fc[:, 1024:],
                                        op=ALU.max)
                nc.vector.tensor_tensor(fb[:, 0:512], fa[:, 0:512], fa[:, 512:1024],
                                        op=ALU.max)
                nc.vector.tensor_tensor(fc[:, 0:256], fb[:, 0:256], fb[:, 256:512],
                                        op=ALU.max)
                mm_t = stats.tile([P, 1], f32)     # m = max(ebuf) (exact in bf16)
                nc.vector.tensor_reduce(mm_t[:], fc[:, 0:256],
                                        axis=mybir.AxisListType.X, op=ALU.max)
                # c = ln(m)/m
                lnm = stats.tile([P, 1], f32)
                nc.scalar.activation(lnm[:], mm_t[:], ACTF.Ln)
                rec = stats.tile([P, 1], f32)
                nc.vector.reciprocal(rec[:], mm_t[:])
                cvec = stats.tile([P, 1], f32)
                nc.vector.tensor_tensor(cvec[:], lnm[:], rec[:], op=ALU.mult)

                obuf = out_pool.tile([P, N_TRAIN], f16)
                for h in range(2):
                    hs = slice(h * (N_TRAIN // 2), (h + 1) * (N_TRAIN // 2))
                    if m in MULT_ON_ACT:
                        nc.scalar.activation(obuf[:, hs], ebuf[:, hs], ACTF.Copy,
                                             bias=0.0, scale=cvec[:])
                    else:
                        nc.vector.tensor_scalar(obuf[:, hs], ebuf[:, hs], cvec[:],
                                                None, op0=ALU.mult)
                    nc.sync.dma_start(out=out_ext[ms, hs], in_=obuf[:, hs])

    nc.compile()
    return nc


_NC = None


def _get_nc():
    global _NC
    if _NC is None:
        _NC = _build()
    return _NC


def kernel(bandwidths, test_Xs, train_Xs):
    bandwidths = np.ascontiguousarray(bandwidths, dtype=np.float32)
    test_Xs = np.ascontiguousarray(test_Xs, dtype=np.float32)
    train_Xs = np.ascontiguousarray(train_Xs, dtype=np.float32)

    nc = _get_nc()
    in_maps = [
        {
            "bandwidths": bandwidths,
            "test_Xs": np.ascontiguousarray(test_Xs[i * M_LOC:(i + 1) * M_LOC]),
            "train_Xs": train_Xs,
        }
        for i in range(N_CORES)
    ]
    res = run_bass_kernel_spmd(nc, in_maps, core_ids=list(range(N_CORES)))
    return np.concatenate([np.asarray(r["out"]).astype(np.float32)
                           for r in res.results], axis=0)


if __name__ == "__main__":
    rng = np.random.default_rng(0)
    h = rng.uniform(0.05, 0.5, N_FEAT).astype(np.float32)
    t = rng.integers(0, 4, (N_TEST, N_FEAT)).astype(np.float32)
    s = rng.integers(0, 4, (N_TRAIN, N_FEAT)).astype(np.float32)
    out = kernel(bandwidths=h, test_Xs=t, train_Xs=s)
    print(out.shape, out.dtype)


# revision 31
# speedup vs baseline: 1.6084x; 1.1956x over previous
"""Aitchison-Aitken categorical kernel on 8 TRN2 NeuronCores.

Math (reference, NUM_LEVELS=4, n_feat=64):
    w_f     = log(1-h_f) - log(h_f/3)
    base    = sum_f log(h_f/3) - sum_f log(h_f) = -64*log(3)   (data independent)
    match   = sum_f w_f * 1[test_if == train_jf]           ([n_test, n_train])
    ld      = match + base
    out     = rowmax(ld) * exp(ld - rowmax(ld))

Device algorithm (per core, data-parallel over test rows):
  - encode test/train as fp16 level-indicator matrices, K = 4*64 = 256
    (2 K-tiles of 128: [lvl0|lvl1], [lvl2|lvl3], feature-duplicated halves);
    weights w folded into the test-side encoding
  - match tile: 2x fp16 matmul accumulation in PSUM (TensorE)
  - ebuf = exp(match + base) via ScalarE straight from PSUM -> bf16
    (no max subtraction needed: exp(ld) spans ~[e^-150, e^41], f32/bf16-safe)
  - row max of ebuf via a bf16 tensor_tensor max fold ladder (2x DVE mode)
  - c = ln(m)/m  (identity: m = e^maxld  =>  c*e^ld = maxld*e^{ld-maxld})
  - out = ebuf * c  (tensor_scalar per-partition; some tiles on ScalarE)

Sharding: test_Xs rows across 8 cores; bandwidths/train_Xs replicated;
out [1024, 8192] local per core, host-concatenated.
"""
import numpy as np
from contextlib import ExitStack

from concourse import bacc, hw_specs, mybir, masks, tile
from concourse.bass_utils import run_bass_kernel_spmd

f32 = mybir.dt.float32
f16 = mybir.dt.float16
bf16 = mybir.dt.bfloat16
ACTF = mybir.ActivationFunctionType
ALU = mybir.AluOpType

N_CORES = 8
N_TEST, N_TRAIN, N_FEAT = 8192, 8192, 64
M_LOC = N_TEST // N_CORES          # 1024 test rows per core
P = 128                            # partitions
M_TILES = M_LOC // P               # 8
NT = 512                           # train cols per matmul (one PSUM bank)
QCOL = 2048                       # psum tile / epilogue chunk (4 banks)
NQ = N_TRAIN // QCOL               # 4
BASE = float(-N_FEAT * np.log(3.0))

# which of the 8 M-tiles get their final multiply on ScalarE
MULT_ON_ACT = set()
GP_FOLD = False

# ---------------------------------------------------------------------------
# Activation-table patch: both Exp and Ln are needed; the stock fixpoint pass
# resolves each to the first table set containing it (exp_and_others vs
# natural_log), reloading the ACT tables (~1.3us) on every switch. Hide
# Exp/Ln from every other set so both resolve to natural_log_exp_and_others
# at its true act_func_set_id (list order/indices preserved).
_COMBINED = "natural_log_exp_and_others"
_orig_get_tables = hw_specs.get_activation_tables


def _patched_tables(module_arch):
    d = _orig_get_tables(module_arch)
    if _COMBINED not in d:
        return d
    hide = {ACTF.Exp, ACTF.Ln}
    return {k: (set(v) if k == _COMBINED else set(v) - hide)
            for k, v in d.items()}


hw_specs.get_activation_tables = _patched_tables
bacc.get_activation_tables = _patched_tables   # bacc imported its own ref
# ---------------------------------------------------------------------------


def _build():
    nc = bacc.Bacc(None, target_bir_lowering=False)
    bw_ext = nc.declare_dram_parameter("bandwidths", [N_FEAT], f32, isOutput=False)
    test_ext = nc.declare_dram_parameter("test_Xs", [M_LOC, N_FEAT], f32, isOutput=False)
    train_ext = nc.declare_dram_parameter("train_Xs", [N_TRAIN, N_FEAT], f32, isOutput=False)
    out_ext = nc.declare_dram_parameter("out", [M_LOC, N_TRAIN], f16, isOutput=True)

    with tile.TileContext(nc) as tc, ExitStack() as ctx:
        const = ctx.enter_context(tc.tile_pool(name="const", bufs=1))
        mm_ps = ctx.enter_context(tc.tile_pool(name="mm_ps", bufs=3, space="PSUM"))
        enc = ctx.enter_context(tc.tile_pool(name="enc", bufs=1))
        stats = ctx.enter_context(tc.tile_pool(name="stats", bufs=4))
        fold_p = ctx.enter_context(tc.tile_pool(name="fold", bufs=1))
        ebuf_pool = ctx.enter_context(tc.tile_pool(name="ebuf", bufs=3))
        out_pool = ctx.enter_context(tc.tile_pool(name="obuf", bufs=2))

        # ---- constants -------------------------------------------------
        ident16 = const.tile([P, P], f16)
        masks.make_identity(nc, ident16[:])

        one_t = const.tile([N_FEAT, 1], f32)
        nc.vector.memset(one_t[:], 1.0)
        base_t = const.tile([P, 1], f32)
        nc.vector.memset(base_t[:], BASE)
        lvlA = const.tile([P, 1], f32)
        nc.vector.memset(lvlA[0:64, :], 0.0)
        nc.vector.memset(lvlA[64:128, :], 1.0)
        lvlB = const.tile([P, 1], f32)
        nc.vector.memset(lvlB[0:64, :], 2.0)
        nc.vector.memset(lvlB[64:128, :], 3.0)

        # ---- w vector from bandwidths ---------------------------------
        bw = const.tile([N_FEAT, 1], f32)
        nc.sync.dma_start(out=bw[:], in_=bw_ext[:].rearrange("(f o) -> f o", o=1))
        lt = const.tile([N_FEAT, 1], f32)   # log(1 - h)
        nc.scalar.activation(lt[:], bw[:], ACTF.Ln, bias=one_t[:], scale=-1.0)
        w2 = const.tile([P, 1], f32)
        lf = const.tile([N_FEAT, 1], f32)   # log(h/3)
        nc.scalar.activation(lf[:], bw[:], ACTF.Ln, scale=1.0 / 3.0)
        nc.vector.tensor_tensor(w2[0:64, :], lt[:], lf[:], op=ALU.subtract)
        nc.sync.dma_start(out=w2[64:128, :], in_=w2[0:64, :])

        # ---- encode: transpose + level indicators ----------------------
        sencA = enc.tile([P, N_TRAIN], f16)
        sencB = enc.tile([P, N_TRAIN], f16)
        tencA = enc.tile([P, M_LOC], f16)
        tencB = enc.tile([P, M_LOC], f16)

        with tc.tile_pool(name="prep", bufs=3) as prep, \
             tc.tile_pool(name="natp", bufs=1) as natp, \
             tc.tile_pool(name="prep_ps", bufs=2, space="PSUM") as prep_ps:

            def transpose_encode(src_ext, n_rows, dstA, dstB, wmul):
                n_ch = n_rows // P              # 128-row chunks
                # feature-duplicated transposed fp16 values [128, n_rows]
                dstT = natp.tile([P, n_rows], f16, tag=f"dstT_{n_rows}")
                # stage + cast natural-layout fp16 copy, 2048 rows at a time
                nat16 = natp.tile([P, n_ch * N_FEAT], f16, tag=f"nat16_{n_rows}")
                row0 = 0
                if n_ch > 8:
                    groups = [1, 1, 2, 4] + [8] * ((n_ch - 8) // 8)
                else:
                    groups = [n_ch]
                for gch in groups:
                    rows = gch * P
                    cpp = gch * N_FEAT
                    col0 = row0 * N_FEAT // P
                    stage = prep.tile([P, cpp], f32, tag="stage")
                    nc.sync.dma_start(
                        out=stage[:].rearrange("p (c f) -> p c f", f=N_FEAT),
                        in_=src_ext[row0:row0 + rows, :].rearrange(
                            "(c p) f -> p c f", p=P))
                    nc.vector.tensor_copy(nat16[:, col0:col0 + cpp], stage[:])
                    row0 += rows
                n_grp = n_ch // 4               # 4 chunks -> one [64, 512] psum
                for g in range(n_grp):
                    pt = prep_ps.tile([64, NT], f16)
                    for j in range(4):
                        c = g * 4 + j
                        cs = slice(c * N_FEAT, (c + 1) * N_FEAT)
                        nc.tensor.transpose(pt[:, j * P:(j + 1) * P],
                                            nat16[:, cs], ident16[:])
                    # evacuate psum -> fp16 top half
                    nc.scalar.activation(dstT[0:64, g * NT:(g + 1) * NT], pt[:],
                                         ACTF.Copy, bias=0.0, scale=1.0)
                # dup + encode: fine chunks first (fast pipeline light-up),
                # coarse later (less DVE op overhead)
                if n_rows > 2048:
                    bounds = [0, 512, 1024, 2048, 4096, 6144, 8192]
                else:
                    bounds = [0, 512, 1024]
                for q in range(len(bounds) - 1):
                    s = slice(bounds[q], min(bounds[q + 1], n_rows))
                    nc.gpsimd.dma_start(out=dstT[64:128, s], in_=dstT[0:64, s])
                    if wmul is None:
                        nc.vector.tensor_scalar(dstA[:, s], dstT[:, s], lvlA[:], None,
                                                op0=ALU.is_equal)
                        nc.vector.tensor_scalar(dstB[:, s], dstT[:, s], lvlB[:], None,
                                                op0=ALU.is_equal)
                    else:
                        nc.vector.tensor_scalar(dstA[:, s], dstT[:, s], lvlA[:], wmul[:],
                                                op0=ALU.is_equal, op1=ALU.mult)
                        nc.vector.tensor_scalar(dstB[:, s], dstT[:, s], lvlB[:], wmul[:],
                                                op0=ALU.is_equal, op1=ALU.mult)

            transpose_encode(test_ext, M_LOC, tencA, tencB, w2)
            transpose_encode(train_ext, N_TRAIN, sencA, sencB, None)

        # ---- main loop --------------------------------------------------
        if True:
            for m in range(M_TILES):
                ms = slice(m * P, (m + 1) * P)
                ebuf = ebuf_pool.tile([P, N_TRAIN], bf16)
                fa = fold_p.tile([P, N_TRAIN // 4], bf16, tag="fa")
                fb = fold_p.tile([P, N_TRAIN // 4], bf16, tag="fb")
                PS_COL = 1024
                for q in range(N_TRAIN // PS_COL):
                    ps = mm_ps.tile([P, PS_COL], f32)
                    # stationary-reuse order: A,A then B,B
                    for j in range(2):
                        n = q * 2 + j
                        nc.tensor.matmul(ps[:, j * NT:(j + 1) * NT], tencA[:, ms],
                                         sencA[:, n * NT:(n + 1) * NT],
                                         start=True, stop=False)
                    for j in range(2):
                        n = q * 2 + j
                        nc.tensor.matmul(ps[:, j * NT:(j + 1) * NT], tencB[:, ms],
                                         sencB[:, n * NT:(n + 1) * NT],
                                         start=False, stop=True)
                    qs = slice(q * PS_COL, (q + 1) * PS_COL)
                    nc.scalar.activation(ebuf[:, qs], ps[:], ACTF.Exp,
                                         bias=base_t[:], scale=1.0)
                    # first-level fold as soon as half the chunks are ready
                    fold_eng = nc.gpsimd if GP_FOLD else nc.vector
                    if q == 3:
                        fold_eng.tensor_tensor(fa[:], ebuf[:, 0:QCOL],
                                               ebuf[:, QCOL:2 * QCOL], op=ALU.max)
                    elif q == 7:
                        fold_eng.tensor_tensor(fb[:], ebuf[:, 2 * QCOL:3 * QCOL],
                                               ebuf[:, 3 * QCOL:], op=ALU.max)
                # fold ladder tail (2x bf16 TT mode, ping-pong, no aliasing)
                fc = fold_p.tile([P, N_TRAIN // 4], bf16, tag="fc")
                nc.vector.tensor_tensor(fc[:], fa[:], fb[:], op=ALU.max)
                nc.vector.tensor_tensor(fa[:, 0:1024], fc[:, 0:1024], fc[:, 1024:],
                                        op=ALU.max)
                nc.vector.tensor_tensor(fb[:, 0:512], fa[:, 0:512], fa[:, 512:1024],
                                        op=ALU.max)
                nc.vector.tensor_tensor(fc[:, 0:256], fb[:, 0:256], fb[:, 256:512],
                                        op=ALU.max)
                mm_t = stats.tile([P, 1], f32)     # m = max(ebuf) (exact in bf16)
                nc.vector.tensor_reduce(mm_t[:], fc[:, 0:256],
                                        axis=mybir.AxisListType.X, op=ALU.max)
                # c = ln(m)/m
                lnm = stats.tile([P, 1], f32)
                nc.scalar.activation(lnm[:], mm_t[:], ACTF.Ln)
                rec = stats.tile([P, 1], f32)
                nc.vector.reciprocal(rec[:], mm_t[:])
                cvec = stats.tile([P, 1], f32)
                nc.vector.tensor_tensor(cvec[:], lnm[:], rec[:], op=ALU.mult)

                obuf = out_pool.tile([P, N_TRAIN], f16)
                n_h = 4 if m == M_TILES - 1 else 2
                hw = N_TRAIN // n_h
                for h in range(n_h):
                    hs = slice(h * hw, (h + 1) * hw)
                    nc.vector.tensor_scalar(obuf[:, hs], ebuf[:, hs], cvec[:],
                                            None, op0=ALU.mult)
                    nc.sync.dma_start(out=out_ext[ms, hs], in_=obuf[:, hs])

    nc.compile()
    return nc


_NC = None


def _get_nc():
    global _NC
    if _NC is None:
        _NC = _build()
    return _NC


def kernel(bandwidths, test_Xs, train_Xs):
    bandwidths = np.ascontiguousarray(bandwidths, dtype=np.float32)
    test_Xs = np.ascontiguousarray(test_Xs, dtype=np.float32)
    train_Xs = np.ascontiguousarray(train_Xs, dtype=np.float32)

    nc = _get_nc()
    in_maps = [
        {
            "bandwidths": bandwidths,
            "test_Xs": np.ascontiguousarray(test_Xs[i * M_LOC:(i + 1) * M_LOC]),
            "train_Xs": train_Xs,
        }
        for i in range(N_CORES)
    ]
    res = run_bass_kernel_spmd(nc, in_maps, core_ids=list(range(N_CORES)))
    return np.concatenate([np.asarray(r["out"]).astype(np.float32)
                           for r in res.results], axis=0)


if __name__ == "__main__":
    rng = np.random.default_rng(0)
    h = rng.uniform(0.05, 0.5, N_FEAT).astype(np.float32)
    t = rng.integers(0, 4, (N_TEST, N_FEAT)).astype(np.float32)
    s = rng.integers(0, 4, (N_TRAIN, N_FEAT)).astype(np.float32)
    out = kernel(bandwidths=h, test_Xs=t, train_Xs=s)
    print(out.shape, out.dtype)
